# revision 3
# baseline (speedup 1.0000x reference)
"""Multi-head attention Bass kernel for Trainium2, SPMD over 8 NeuronCores.

Problem (hardcoded): B=2, L=2048, D=1024, H=16, HD=64, fp32.
    q/k/v = per-head projections of x with shared Wq/Wk/Wv (64x64)
    scores = softmax(mask(q @ k^T) / 8), attn = scores @ v
    out = concat(attn) @ Wo.T + bo

Sharding: data-parallel over batch (2) x query-parallel (4) = 8 cores.
Each core computes the full attention for a 512-query slice of one batch
element (K/V computed over the full sequence on-core; no collectives),
then its slice of the output projection. Host concatenates slices.

Device layout (per core) — everything transposed so softmax reduction
lands on PE matmuls and elementwise ops stay on the free axis:
    xT   [1024, 2048]  x[b].T (features on partitions, per 128-row tiles)
    QT̂   [64, 512]     (0.125 * Wq) @ X_h^T        (scale folded into Wq)
    KT   [64, 2048]    Wk @ X_h^T
    V    [128c, 64]    X_h @ Wv.T   (natural layout, 16 chunks of 128 keys)
    S̃T   [128c, 512]   KT_chunk.T @ QT̂  = (scores.T)/8
    P̂T   = exp(S̃T) * M01T   (multiplicative 0/1 mask; no max-subtraction —
                              logits are tiny, exp is safe)
    attnT[65, 512]     per head: rows 0-63 = V.T @ P̂T, row 64 = sum_k P̂T
                       (ones column appended to V gives the softmax
                        denominator for free)
    out  [512, 1024]   attnT.T @ Wo.T + bo, accumulated over hd chunks
"""

import numpy as np

B, L, D, H, HD = 2, 2048, 1024, 16, 64
NCORES = 8
QS = L // 4  # 512 queries per core
NCH = L // 128  # 16 key chunks

_cache = {}


def _emit(tc, aps, dt_mm):
    import concourse.bass as bass
    import concourse.mybir as mybir

    nc = tc.nc
    f32 = mybir.dt.float32
    Exp = mybir.ActivationFunctionType.Exp

    def mm(ap):  # matmul-operand view (float32 or float32r)
        return ap.bitcast(dt_mm) if dt_mm != f32 else ap

    xT_d, xTq_d, m01_d, wq_d, wk_d, wv_d, woT_d, bo_d, out_d = aps

    import contextlib

    with contextlib.ExitStack() as ctx:
        # ---- persistent SBUF ----
        const_pool = ctx.enter_context(tc.tile_pool(name="const", bufs=1))
        xtq_sb = const_pool.tile([128, 8 * QS], f32, tag="xtq")
        m01_sb = const_pool.tile([128, NCH * QS], f32, tag="m01")
        woT_sb = const_pool.tile([128, 8 * 1024], f32, tag="woT")
        wq_sb = const_pool.tile([128, 64], f32, tag="wq")
        wk_sb = const_pool.tile([128, 64], f32, tag="wk")
        wv_sb = const_pool.tile([128, 64], f32, tag="wv")
        bo_sb = const_pool.tile([1, 1024], f32, tag="bo")
        ones64 = const_pool.tile([1, 64], f32, tag="ones64")
        onesq = const_pool.tile([1, 128], f32, tag="onesq")
        # V chunks augmented with a ones column; 2 parity buffers
        vones = const_pool.tile([128, 2 * NCH * 65], f32, tag="vones")
        attnT_sb = const_pool.tile([128, 8 * QS], f32, tag="attnT")

        nc.sync.dma_start(
            out=xtq_sb[:].rearrange("p (t q) -> p t q", t=8),
            in_=xTq_d.rearrange("(t p) q -> p t q", p=128),
        )
        nc.sync.dma_start(out=m01_sb[:], in_=m01_d)
        for dc in range(8):
            nc.sync.dma_start(
                out=woT_sb[:, 1024 * dc : 1024 * (dc + 1)],
                in_=woT_d[128 * dc : 128 * (dc + 1), :],
            )
        nc.sync.dma_start(out=wq_sb[:], in_=wq_d)
        nc.sync.dma_start(out=wk_sb[:], in_=wk_d)
        nc.sync.dma_start(out=wv_sb[:], in_=wv_d)
        nc.sync.dma_start(out=bo_sb[:], in_=bo_d)
        nc.vector.memset(ones64[:], 1.0)
        nc.vector.memset(onesq[:], 1.0)
        nc.vector.memset(vones[:], 1.0)  # ones cols live at [.., 64]; V overwrites 0:63

        # ---- working pools ----
        xt_pool = ctx.enter_context(tc.tile_pool(name="xt", bufs=2))
        qt_pool = ctx.enter_context(tc.tile_pool(name="qt", bufs=2))
        kt_pool = ctx.enter_context(tc.tile_pool(name="kt", bufs=2))
        pt_pool = ctx.enter_context(tc.tile_pool(name="pt", bufs=1))
        rb_pool = ctx.enter_context(tc.tile_pool(name="rb", bufs=2))
        r_pool = ctx.enter_context(tc.tile_pool(name="r", bufs=2))

        ps_qkv = ctx.enter_context(tc.tile_pool(name="ps_qkv", bufs=2, space="PSUM"))
        ps_sm = ctx.enter_context(tc.tile_pool(name="ps_sm", bufs=2, space="PSUM"))
        ps_ap = ctx.enter_context(tc.tile_pool(name="ps_ap", bufs=2, space="PSUM"))

        xt = None
        for h in range(H):
            hi, po = h // 2, 64 * (h % 2)
            par = h % 2
            if po == 0:
                xt = xt_pool.tile([128, L], f32, tag="xt")
                nc.sync.dma_start(out=xt[:], in_=xT_d[128 * hi : 128 * (hi + 1), :])

            # QT̂ = (0.125*Wq) @ X_h^T  (q slice only)
            qt_ps = ps_qkv.tile([64, QS], f32, tag="qkv")
            nc.tensor.matmul(
                out=qt_ps[:],
                lhsT=mm(wq_sb[po : po + 64, :]),
                rhs=mm(xtq_sb[po : po + 64, QS * hi : QS * (hi + 1)]),
                start=True,
                stop=True,
            )
            qt_sb = qt_pool.tile([64, QS], f32, tag="qt")
            nc.scalar.copy(out=qt_sb[:], in_=qt_ps[:])

            # KT = Wk @ X_h^T (full L)
            kt_sb = kt_pool.tile([64, L], f32, tag="kt")
            for j in range(4):
                kt_ps = ps_qkv.tile([64, 512], f32, tag="qkv")
                nc.tensor.matmul(
                    out=kt_ps[:],
                    lhsT=mm(wk_sb[po : po + 64, :]),
                    rhs=mm(xt[po : po + 64, 512 * j : 512 * (j + 1)]),
                    start=True,
                    stop=True,
                )
                nc.scalar.copy(out=kt_sb[:, 512 * j : 512 * (j + 1)], in_=kt_ps[:])

            # V chunks (natural [k, d]) -> vones, 8 chunks per PSUM bank
            for half in range(2):
                v_ps = ps_qkv.tile([128, 512], f32, tag="qkv")
                for cc in range(8):
                    c = 8 * half + cc
                    nc.tensor.matmul(
                        out=v_ps[:, 64 * cc : 64 * (cc + 1)],
                        lhsT=mm(xt[po : po + 64, 128 * c : 128 * (c + 1)]),
                        rhs=mm(wv_sb[po : po + 64, :]),
                        start=True,
                        stop=True,
                    )
                # copy to vones cols [ (par*NCH+c)*65 .. +64 )
                vo = vones[:].rearrange("p (n m) -> p n m", m=65)
                nc.vector.tensor_copy(
                    out=vo[:, par * NCH + 8 * half : par * NCH + 8 * (half + 1), 0:64],
                    in_=v_ps[:].rearrange("p (n m) -> p n m", m=64),
                )

            # S̃T chunks -> exp -> P̂T
            pt_sb = pt_pool.tile([128, NCH * QS], f32, tag="pt")
            for cp in range(8):  # chunk pairs
                sm_ps = ps_sm.tile([128, 2 * QS], f32, tag="sm")
                for k in range(2):
                    c = 2 * cp + k
                    nc.tensor.matmul(
                        out=sm_ps[:, QS * k : QS * (k + 1)],
                        lhsT=mm(kt_sb[:, 128 * c : 128 * (c + 1)]),
                        rhs=mm(qt_sb[:]),
                        start=True,
                        stop=True,
                    )
                nc.scalar.activation(
                    out=pt_sb[:, 2 * QS * cp : 2 * QS * (cp + 1)], in_=sm_ps[:], func=Exp
                )
            # mask (multiplicative 0/1)
            for g in range(4):
                sl = slice(4 * QS * g, 4 * QS * (g + 1))
                nc.vector.tensor_mul(out=pt_sb[:, sl], in0=pt_sb[:, sl], in1=m01_sb[:, sl])

            # attnT accumulation: [65, 512], row 64 = denominator
            ap_ps = ps_ap.tile([65, QS], f32, tag="ap")
            vo = vones[:].rearrange("p (n m) -> p n m", m=65)
            for c in range(NCH):
                nc.tensor.matmul(
                    out=ap_ps[:],
                    lhsT=mm(vo[:, par * NCH + c, :]),
                    rhs=mm(pt_sb[:, QS * c : QS * (c + 1)]),
                    start=(c == 0),
                    stop=(c == NCH - 1),
                )

            # normalize: attnT_h = ap[0:64] * (1/denom) broadcast over partitions
            r_sb = r_pool.tile([1, QS], f32, tag="r")
            nc.vector.reciprocal(out=r_sb[:], in_=ap_ps[64:65, :])
            rb_sb = rb_pool.tile([64, QS], f32, tag="rb")
            nc.gpsimd.partition_broadcast(rb_sb[:], r_sb[:])
            nc.vector.tensor_mul(
                out=attnT_sb[po : po + 64, QS * hi : QS * (hi + 1)],
                in0=ap_ps[0:64, :],
                in1=rb_sb[:],
            )

    # ---- output projection: out[q, e] = attnT.T @ WoT + bo ----
    with contextlib.ExitStack() as ctx:
        ps_op = ctx.enter_context(tc.tile_pool(name="ps_op", bufs=2, space="PSUM"))
        ob_pool = ctx.enter_context(tc.tile_pool(name="ob", bufs=2))
        for qc in range(4):
            op_ps = ps_op.tile([128, 1024], f32, tag="op")
            for eh in range(2):
                for dc in range(8):
                    nc.tensor.matmul(
                        out=op_ps[:, 512 * eh : 512 * (eh + 1)],
                        lhsT=mm(attnT_sb[:, QS * dc + 128 * qc : QS * dc + 128 * (qc + 1)]),
                        rhs=mm(woT_sb[:, 1024 * dc + 512 * eh : 1024 * dc + 512 * (eh + 1)]),
                        start=(dc == 0),
                        stop=False,
                    )
                nc.tensor.matmul(
                    out=op_ps[:, 512 * eh : 512 * (eh + 1)],
                    lhsT=mm(onesq[:]),
                    rhs=mm(bo_sb[:, 512 * eh : 512 * (eh + 1)]),
                    start=False,
                    stop=True,
                )
            out_sb = ob_pool.tile([128, 1024], f32, tag="ob")
            nc.vector.tensor_copy(out=out_sb[:], in_=op_ps[:])
            nc.sync.dma_start(out=out_d[128 * qc : 128 * (qc + 1), :], in_=out_sb[:])


def _build(dt_mm_name):
    import concourse.bacc as bacc
    import concourse.mybir as mybir
    import concourse.tile as tile

    f32 = mybir.dt.float32
    dt_mm = getattr(mybir.dt, dt_mm_name)
    nc = bacc.Bacc("TRN2", target_bir_lowering=False, debug=False)
    t = lambda name, shape, kind: nc.dram_tensor(name, shape, f32, kind=kind).ap()
    aps = (
        t("xT", (D, L), "ExternalInput"),
        t("xTq", (D, QS), "ExternalInput"),
        t("m01", (128, NCH * QS), "ExternalInput"),
        t("wq", (128, 64), "ExternalInput"),
        t("wk", (128, 64), "ExternalInput"),
        t("wv", (128, 64), "ExternalInput"),
        t("woT", (D, D), "ExternalInput"),
        t("bo", (1, D), "ExternalInput"),
        t("out", (QS, D), "ExternalOutput"),
    )
    with tile.TileContext(nc) as tc:
        _emit(tc, aps, dt_mm)
    nc.compile()
    return nc


def get_nc(dt_mm_name="float32r"):
    if dt_mm_name not in _cache:
        _cache[dt_mm_name] = _build(dt_mm_name)
    return _cache[dt_mm_name]


def _host_prep(x, padding_mask, future_mask, Wq, Wk, Wv, Wo, bo):
    x = np.asarray(x, np.float32)
    fm = np.asarray(future_mask, np.int64)
    pm = np.asarray(padding_mask, np.int64)
    xT = np.ascontiguousarray(x.transpose(0, 2, 1))  # (B, D, L)
    # masked where future+padding > 1 -> multiplicative 0; else 1
    m01 = ((fm[None, :, :] + pm[:, None, :]) <= 1).astype(np.float32)  # (B, q, k)
    m01T = np.ascontiguousarray(m01.transpose(0, 2, 1))  # (B, k, q)
    wq = np.concatenate([Wq.T * 0.125] * 2, 0).astype(np.float32)
    wk = np.concatenate([Wk.T] * 2, 0).astype(np.float32)
    wv = np.concatenate([Wv.T] * 2, 0).astype(np.float32)
    woT = np.ascontiguousarray(np.asarray(Wo, np.float32).T)
    bo2 = np.asarray(bo, np.float32).reshape(1, D)
    in_maps = []
    for core in range(NCORES):
        b, qo = core // 4, QS * (core % 4)
        m = m01T[b][:, qo : qo + QS]  # (2048, 512)
        m_dev = np.ascontiguousarray(
            m.reshape(NCH, 128, QS).transpose(1, 0, 2).reshape(128, NCH * QS)
        )
        in_maps.append(
            {
                "xT": xT[b],
                "xTq": np.ascontiguousarray(xT[b][:, qo : qo + QS]),
                "m01": m_dev,
                "wq": wq,
                "wk": wk,
                "wv": wv,
                "woT": woT,
                "bo": bo2,
            }
        )
    return in_maps


def run(inputs_dict, dt_mm_name="float32r", **spmd_kwargs):
    from concourse.bass_utils import run_bass_kernel_spmd

    nc = get_nc(dt_mm_name)
    in_maps = _host_prep(**inputs_dict)
    res = run_bass_kernel_spmd(nc, in_maps, core_ids=list(range(NCORES)), **spmd_kwargs)
    out = np.empty((B, L, D), np.float32)
    for core in range(NCORES):
        b, qo = core // 4, QS * (core % 4)
        out[b, qo : qo + QS, :] = res.results[core]["out"]
    return out, res


def kernel(**inputs):
    out, _ = run(inputs)
    return out


# revision 5
# speedup vs baseline: 2.2573x; 2.2573x over previous
"""Multi-head attention Bass kernel for Trainium2, SPMD over 8 NeuronCores.

Problem (hardcoded): B=2, L=2048, D=1024, H=16, HD=64, fp32.
    q/k/v = per-head projections of x with shared Wq/Wk/Wv (64x64)
    scores = softmax(mask(q @ k^T) / 8), attn = scores @ v
    out = concat(attn) @ Wo.T + bo

Sharding: data-parallel over batch (2) x query-parallel (4) = 8 cores.
Each core computes the full attention for a 512-query slice of one batch
element (K/V computed over the full sequence on-core; no collectives),
then its slice of the output projection. Host concatenates slices.

Device layout (per core) — everything transposed so softmax reduction
lands on PE matmuls and elementwise ops stay on the free axis:
    xT   [1024, 2048]  x[b].T (features on partitions, per 128-row tiles)
    QT̂   [64, 512]     (0.125 * Wq) @ X_h^T        (scale folded into Wq)
    KT   [64, 2048]    Wk @ X_h^T
    V    [128c, 64]    X_h @ Wv.T   (natural layout, 16 chunks of 128 keys)
    S̃T   [128c, 512]   KT_chunk.T @ QT̂  = (scores.T)/8
    P̂T   = exp(S̃T) * M01T   (multiplicative 0/1 mask; no max-subtraction —
                              logits are tiny, exp is safe)
    attnT[65, 512]     per head: rows 0-63 = V.T @ P̂T, row 64 = sum_k P̂T
                       (ones column appended to V gives the softmax
                        denominator for free)
    out  [512, 1024]   attnT.T @ Wo.T + bo, accumulated over hd chunks
"""

import numpy as np

B, L, D, H, HD = 2, 2048, 1024, 16, 64
NCORES = 8
QS = L // 4  # 512 queries per core
NCH = L // 128  # 16 key chunks

_cache = {}


def _emit(tc, aps, dt_mm):
    import concourse.bass as bass
    import concourse.mybir as mybir

    nc = tc.nc
    f32 = mybir.dt.float32
    Exp = mybir.ActivationFunctionType.Exp

    dmm = dt_mm  # dtype for every tensor feeding a matmul

    def mm(ap):
        return ap

    xT_d, xTq_d, m01_d, wq_d, wk_d, wv_d, woT_d, bo_d, ones_d, out_d = aps

    import contextlib

    with contextlib.ExitStack() as ctx:
        # ---- persistent SBUF ----
        const_pool = ctx.enter_context(tc.tile_pool(name="const", bufs=1))
        xtq_sb = const_pool.tile([128, 8 * QS], dmm, tag="xtq")
        m01_sb = const_pool.tile([128, NCH * QS], dmm, tag="m01")
        woT_sb = const_pool.tile([128, 8 * 1024], dmm, tag="woT")
        wq_sb = const_pool.tile([128, 64], dmm, tag="wq")
        wk_sb = const_pool.tile([128, 64], dmm, tag="wk")
        wv_sb = const_pool.tile([128, 64], dmm, tag="wv")
        bo_sb = const_pool.tile([1, 1024], dmm, tag="bo")
        onesq = const_pool.tile([1, 128], dmm, tag="onesq")
        # V chunks augmented with a ones column; 2 parity buffers
        vones = const_pool.tile([128, 2 * NCH * 65], dmm, tag="vones")
        attnT_sb = const_pool.tile([128, 8 * QS], dmm, tag="attnT")

        nc.sync.dma_start(
            out=xtq_sb[:].rearrange("p (t q) -> p t q", t=8),
            in_=xTq_d.rearrange("(t p) q -> p t q", p=128),
        )
        nc.sync.dma_start(out=m01_sb[:], in_=m01_d)
        for dc in range(8):
            nc.sync.dma_start(
                out=woT_sb[:, 1024 * dc : 1024 * (dc + 1)],
                in_=woT_d[128 * dc : 128 * (dc + 1), :],
            )
        nc.sync.dma_start(out=wq_sb[:], in_=wq_d)
        nc.sync.dma_start(out=wk_sb[:], in_=wk_d)
        nc.sync.dma_start(out=wv_sb[:], in_=wv_d)
        nc.sync.dma_start(out=bo_sb[:], in_=bo_d)
        # ones constants DMA'd from DRAM (memset can't write float32r)
        nc.sync.dma_start(out=onesq[:], in_=ones_d[0:1, :])
        vo0 = vones[:].rearrange("p (n m) -> p n m", m=65)
        nc.sync.dma_start(out=vo0[:, :, 64:65], in_=ones_d[:, 0 : 2 * NCH].unsqueeze(-1))

        # ---- working pools ----
        xt_pool = ctx.enter_context(tc.tile_pool(name="xt", bufs=2))
        qt_pool = ctx.enter_context(tc.tile_pool(name="qt", bufs=2))
        kt_pool = ctx.enter_context(tc.tile_pool(name="kt", bufs=2))
        pt_pool = ctx.enter_context(tc.tile_pool(name="pt", bufs=1))
        rb_pool = ctx.enter_context(tc.tile_pool(name="rb", bufs=2))
        r_pool = ctx.enter_context(tc.tile_pool(name="r", bufs=2))

        ps_qkv = ctx.enter_context(tc.tile_pool(name="ps_qkv", bufs=2, space="PSUM"))
        ps_sm = ctx.enter_context(tc.tile_pool(name="ps_sm", bufs=2, space="PSUM"))
        ps_ap = ctx.enter_context(tc.tile_pool(name="ps_ap", bufs=2, space="PSUM"))

        xt = None
        for h in range(H):
            hi, po = h // 2, 64 * (h % 2)
            par = h % 2
            if po == 0:
                xt = xt_pool.tile([128, L], dmm, tag="xt")
                nc.sync.dma_start(out=xt[:], in_=xT_d[128 * hi : 128 * (hi + 1), :])

            # QT̂ = (0.125*Wq) @ X_h^T  (q slice only)
            qt_ps = ps_qkv.tile([64, QS], f32, tag="qkv")
            nc.tensor.matmul(
                out=qt_ps[:],
                lhsT=mm(wq_sb[po : po + 64, :]),
                rhs=mm(xtq_sb[po : po + 64, QS * hi : QS * (hi + 1)]),
                start=True,
                stop=True,
            )
            qt_sb = qt_pool.tile([64, QS], dmm, tag="qt")
            nc.scalar.copy(out=qt_sb[:], in_=qt_ps[:])

            # KT = Wk @ X_h^T (full L)
            kt_sb = kt_pool.tile([64, L], dmm, tag="kt")
            for j in range(4):
                kt_ps = ps_qkv.tile([64, 512], f32, tag="qkv")
                nc.tensor.matmul(
                    out=kt_ps[:],
                    lhsT=mm(wk_sb[po : po + 64, :]),
                    rhs=mm(xt[po : po + 64, 512 * j : 512 * (j + 1)]),
                    start=True,
                    stop=True,
                )
                nc.scalar.copy(out=kt_sb[:, 512 * j : 512 * (j + 1)], in_=kt_ps[:])

            # V chunks (natural [k, d]) -> vones, 8 chunks per PSUM bank
            for half in range(2):
                v_ps = ps_qkv.tile([128, 512], f32, tag="qkv")
                for cc in range(8):
                    c = 8 * half + cc
                    nc.tensor.matmul(
                        out=v_ps[:, 64 * cc : 64 * (cc + 1)],
                        lhsT=mm(xt[po : po + 64, 128 * c : 128 * (c + 1)]),
                        rhs=mm(wv_sb[po : po + 64, :]),
                        start=True,
                        stop=True,
                    )
                # copy to vones cols [ (par*NCH+c)*65 .. +64 )
                vo = vones[:].rearrange("p (n m) -> p n m", m=65)
                nc.vector.tensor_copy(
                    out=vo[:, par * NCH + 8 * half : par * NCH + 8 * (half + 1), 0:64],
                    in_=v_ps[:].rearrange("p (n m) -> p n m", m=64),
                )

            # S̃T chunks -> exp -> P̂T
            pt_sb = pt_pool.tile([128, NCH * QS], dmm, tag="pt")
            for cp in range(8):  # chunk pairs
                sm_ps = ps_sm.tile([128, 2 * QS], f32, tag="sm")
                for k in range(2):
                    c = 2 * cp + k
                    nc.tensor.matmul(
                        out=sm_ps[:, QS * k : QS * (k + 1)],
                        lhsT=mm(kt_sb[:, 128 * c : 128 * (c + 1)]),
                        rhs=mm(qt_sb[:]),
                        start=True,
                        stop=True,
                    )
                nc.scalar.activation(
                    out=pt_sb[:, 2 * QS * cp : 2 * QS * (cp + 1)], in_=sm_ps[:], func=Exp
                )
            # mask (multiplicative 0/1)
            for g in range(4):
                sl = slice(4 * QS * g, 4 * QS * (g + 1))
                nc.vector.tensor_mul(out=pt_sb[:, sl], in0=pt_sb[:, sl], in1=m01_sb[:, sl])

            # attnT accumulation: [65, 512], row 64 = denominator
            ap_ps = ps_ap.tile([65, QS], f32, tag="ap")
            vo = vones[:].rearrange("p (n m) -> p n m", m=65)
            for c in range(NCH):
                nc.tensor.matmul(
                    out=ap_ps[:],
                    lhsT=mm(vo[:, par * NCH + c, :]),
                    rhs=mm(pt_sb[:, QS * c : QS * (c + 1)]),
                    start=(c == 0),
                    stop=(c == NCH - 1),
                )

            # normalize: attnT_h = ap[0:64] * (1/denom) broadcast over partitions
            r_sb = r_pool.tile([1, QS], f32, tag="r")
            nc.vector.reciprocal(out=r_sb[:], in_=ap_ps[64:65, :])
            rb_sb = rb_pool.tile([64, QS], f32, tag="rb")
            nc.gpsimd.partition_broadcast(rb_sb[:], r_sb[:])
            nc.vector.tensor_mul(
                out=attnT_sb[po : po + 64, QS * hi : QS * (hi + 1)],
                in0=ap_ps[0:64, :],
                in1=rb_sb[:],
            )

    # ---- output projection: out[q, e] = attnT.T @ WoT + bo ----
    with contextlib.ExitStack() as ctx:
        ps_op = ctx.enter_context(tc.tile_pool(name="ps_op", bufs=2, space="PSUM"))
        ob_pool = ctx.enter_context(tc.tile_pool(name="ob", bufs=2))
        for qc in range(4):
            op_ps = ps_op.tile([128, 1024], f32, tag="op")
            for eh in range(2):
                for dc in range(8):
                    nc.tensor.matmul(
                        out=op_ps[:, 512 * eh : 512 * (eh + 1)],
                        lhsT=mm(attnT_sb[:, QS * dc + 128 * qc : QS * dc + 128 * (qc + 1)]),
                        rhs=mm(woT_sb[:, 1024 * dc + 512 * eh : 1024 * dc + 512 * (eh + 1)]),
                        start=(dc == 0),
                        stop=False,
                    )
                nc.tensor.matmul(
                    out=op_ps[:, 512 * eh : 512 * (eh + 1)],
                    lhsT=mm(onesq[:]),
                    rhs=mm(bo_sb[:, 512 * eh : 512 * (eh + 1)]),
                    start=False,
                    stop=True,
                )
            out_sb = ob_pool.tile([128, 1024], f32, tag="ob")
            nc.vector.tensor_copy(out=out_sb[:], in_=op_ps[:])
            nc.sync.dma_start(out=out_d[128 * qc : 128 * (qc + 1), :], in_=out_sb[:])


def _build(dt_mm_name):
    import concourse.bacc as bacc
    import concourse.mybir as mybir
    import concourse.tile as tile

    f32 = mybir.dt.float32
    dt_mm = getattr(mybir.dt, dt_mm_name)
    nc = bacc.Bacc("TRN2", target_bir_lowering=False, debug=False)

    def t(name, shape, kind, dt=dt_mm):
        return nc.dram_tensor(name, shape, dt, kind=kind).ap()
    aps = (
        t("xT", (D, L), "ExternalInput"),
        t("xTq", (D, QS), "ExternalInput"),
        t("m01", (128, NCH * QS), "ExternalInput"),
        t("wq", (128, 64), "ExternalInput"),
        t("wk", (128, 64), "ExternalInput"),
        t("wv", (128, 64), "ExternalInput"),
        t("woT", (D, D), "ExternalInput"),
        t("bo", (1, D), "ExternalInput"),
        t("ones", (128, 128), "ExternalInput"),
        t("out", (QS, D), "ExternalOutput", f32),
    )
    with tile.TileContext(nc) as tc:
        _emit(tc, aps, dt_mm)
    nc.compile()
    return nc


def get_nc(dt_mm_name="float32r"):
    if dt_mm_name not in _cache:
        _cache[dt_mm_name] = _build(dt_mm_name)
    return _cache[dt_mm_name]


def _host_prep(x, padding_mask, future_mask, Wq, Wk, Wv, Wo, bo):
    x = np.asarray(x, np.float32)
    fm = np.asarray(future_mask, np.int64)
    pm = np.asarray(padding_mask, np.int64)
    xT = np.ascontiguousarray(x.transpose(0, 2, 1))  # (B, D, L)
    # masked where future+padding > 1 -> multiplicative 0; else 1
    m01 = ((fm[None, :, :] + pm[:, None, :]) <= 1).astype(np.float32)  # (B, q, k)
    m01T = np.ascontiguousarray(m01.transpose(0, 2, 1))  # (B, k, q)
    wq = np.concatenate([Wq.T * 0.125] * 2, 0).astype(np.float32)
    wk = np.concatenate([Wk.T] * 2, 0).astype(np.float32)
    wv = np.concatenate([Wv.T] * 2, 0).astype(np.float32)
    woT = np.ascontiguousarray(np.asarray(Wo, np.float32).T)
    bo2 = np.asarray(bo, np.float32).reshape(1, D)
    in_maps = []
    for core in range(NCORES):
        b, qo = core // 4, QS * (core % 4)
        m = m01T[b][:, qo : qo + QS]  # (2048, 512)
        m_dev = np.ascontiguousarray(
            m.reshape(NCH, 128, QS).transpose(1, 0, 2).reshape(128, NCH * QS)
        )
        in_maps.append(
            {
                "xT": xT[b],
                "xTq": np.ascontiguousarray(xT[b][:, qo : qo + QS]),
                "m01": m_dev,
                "wq": wq,
                "wk": wk,
                "wv": wv,
                "woT": woT,
                "bo": bo2,
                "ones": np.ones((128, 128), np.float32),
            }
        )
    return in_maps


def run(inputs_dict, dt_mm_name="float32r", **spmd_kwargs):
    from concourse.bass_utils import run_bass_kernel_spmd

    nc = get_nc(dt_mm_name)
    in_maps = _host_prep(**inputs_dict)
    res = run_bass_kernel_spmd(nc, in_maps, core_ids=list(range(NCORES)), **spmd_kwargs)
    out = np.empty((B, L, D), np.float32)
    for core in range(NCORES):
        b, qo = core // 4, QS * (core % 4)
        out[b, qo : qo + QS, :] = res.results[core]["out"]
    return out, res


def kernel(**inputs):
    out, _ = run(inputs)
    return out
